# revision 25
# baseline (speedup 1.0000x reference)
"""Trainium2 Bass kernel for nn_AllAttLayer (cross-batch attention gating layer).

Reference computation (B=8, C=512, H=W=32, HW=1024):
    xf = x as [B, HW, C]
    q = xf @ Wq.T + bq ; k = xf @ Wk.T + bk
    scores = q.flat @ k.flat.T                  # [B*HW, B*HW]
    xw = max over each image's keys, mean over images   # [B*HW]
    xw = softmax(xw * C**-0.5 per image)        # [B, HW]
    out = (x * xw) @ W6.T + b6  (1x1 conv)      # == W6 @ (x * xw)

Key algebraic restructure: scores = q^T k = q^T (Wk x + bk)
  = (Wk^T Wq x + Wk^T bq)^T x  +  (x^T Wq^T bk + bq^T bk).
The host folds Wqk := Wk^T Wq and bqk := Wk^T bq, so the kernel projects
each core's queries ONCE (qt = Wqk x + bqk, fp8 DoubleRow from the
replicated fp8 x) and the score matmuls consume the replicated fp8 x
DIRECTLY as the moving operand -- no per-image key projection, no key
evacuations. The k-bias term x^T(Wq^T bk) =: qbk is a 1-column fp8
projection (host-folded hbk := Wq^T bk) added to the logits; its
constant part bq^T bk is uniform over all queries and cancels in
softmax.

Sharding: core b owns image b (its 1024 queries). No collectives: the
host replicates x in fp8 DoubleRow layout, ROLLED per core so the
core's own image is slot 0 (the kernel is SPMD -- same program, per-core
data). Everything is c-major ([C, HW]) so no transposes are needed.

Engine schedule: per (query-block, image) the two 512-key score halves
land in one paired [128,1024] PSUM tile (2 banks); query blocks 0..NDVE-1
are consumed by a single DVE max-reduce (~1.19us), the rest by a
ScalarE exp-accumulate (LSE max approximation with temperature 2 and a
-80 shift to keep exp sums in range; the ~ln(n_eff)/2 overestimate is
~0.5 on logits*SCALE/8 ~ 0.003, well under the tolerance). Per image:
PE 7.6us of score matmuls vs DVE ~6.0us + scalar ~3.5us -- PE-paced.
The final conv runs bf16 on ungated x DURING the last image's score
drain; the gate (and b6) applies at the tail. fp32 elsewhere.
"""

import sys
import numpy as np

for _p in ("/opt/trn_rl_repo",):
    if _p not in sys.path:
        sys.path.insert(0, _p)

B, C, H, W = 8, 512, 32, 32
HW = H * W              # 1024 pixels per image
NCORES = 8
CB = C // 128           # 4 channel blocks
G = 2                   # DoubleRow groups (K=256 each)
QB = HW // 128          # 8 query blocks per core
KH = 2                  # key halves (512 keys each)
NIMG = NCORES
SCALE = 1.0 / float(np.sqrt(C))

WQK_SCALE = 64.0        # host scales Wqk by this before fp8
HBK_SCALE = 16.0        # host scales hbk by this before fp8
NDVE = 5                # query blocks per image consumed by DVE (rest: LSE)
LSE_T = 2.0             # LSE temperature
LSE_SHIFT = 36.0        # exp(t*s-SHIFT): sums stay in the HW ln
                        # spline's valid range [1e-18, 1e19]


def build_kernel():
    from concourse import bacc, tile, mybir

    f32 = mybir.dt.float32
    bf16 = mybir.dt.bfloat16
    fp8 = mybir.dt.float8e4
    DR = mybir.MatmulPerfMode.DoubleRow

    nc = bacc.Bacc("TRN2", target_bir_lowering=False, debug=False,
                   num_devices=NCORES)

    x_in = nc.dram_tensor("x", [C, HW], bf16, kind="ExternalInput").ap()
    w6t_in = nc.dram_tensor("w6t", [C, C], bf16, kind="ExternalInput").ap()
    x8_in = [nc.dram_tensor(f"x8g{g}", [128, 2 * NCORES * HW], fp8,
                            kind="ExternalInput").ap() for g in range(G)]
    wqk8_in = [nc.dram_tensor(f"wqk8g{g}", [128, 2 * C], fp8,
                              kind="ExternalInput").ap() for g in range(G)]
    hbk8_in = [nc.dram_tensor(f"hbk8g{g}", [128, 2], fp8,
                              kind="ExternalInput").ap() for g in range(G)]
    bqk_in = nc.dram_tensor("bqk", [C, 1], f32, kind="ExternalInput").ap()
    b6_in = nc.dram_tensor("b6", [C, 1], f32, kind="ExternalInput").ap()
    out_ext = nc.dram_tensor("out", [C, HW], f32, kind="ExternalOutput").ap()

    AF = mybir.ActivationFunctionType
    ALU = mybir.AluOpType
    AX = mybir.AxisListType

    def dr3(ap, span):
        """[128, G*span] tile AP -> [128, 2, span] DoubleRow view."""
        return ap.rearrange("p (i n) -> p i n", i=2, n=span)

    with tile.TileContext(nc) as tc:
        with tc.tile_pool(name="consts", bufs=1) as consts, \
             tc.tile_pool(name="wpool", bufs=1) as wpool, \
             tc.tile_pool(name="xpool", bufs=1) as xpool, \
             tc.tile_pool(name="qpool", bufs=1) as qpool, \
             tc.tile_pool(name="redpool", bufs=1) as redpool, \
             tc.tile_pool(name="scrpool", bufs=3) as scrpool, \
             tc.tile_pool(name="outpool", bufs=1) as outpool, \
             tc.tile_pool(name="dram", bufs=1, space="DRAM") as dram, \
             tc.tile_pool(name="ps", bufs=4, space="PSUM") as psp:

            bias_sb = {}

            def load_bias(nm, src, eng):
                t = consts.tile([128, CB], f32, tag=f"{nm}_sb", name=f"{nm}_sb")
                for co in range(CB):
                    eng.dma_start(out=t[:, co:co + 1],
                                  in_=src[co * 128:(co + 1) * 128, :])
                bias_sb[nm] = t

            # ---- head loads ----
            # own-image x8 slices (slot 0) and Wqk first (they gate the
            # qt projection); the scalar queue gets no head DMAs.
            x8_sb = []
            for g in range(G):
                t = xpool.tile([128, 2 * NCORES * HW], fp8, tag=f"x8{g}",
                               name=f"x8{g}")
                x8_sb.append(t)
            wqk8_sb, hbk8_sb = [], []
            for g in range(G):
                t = wpool.tile([128, 2 * C], fp8, tag=f"wqk8{g}",
                               name=f"wqk8{g}")
                nc.sync.dma_start(out=t[:], in_=wqk8_in[g][:])
                wqk8_sb.append(t)
            for g in range(G):
                for i in range(2):
                    c0 = i * NCORES * HW
                    eng = nc.sync if (g + i) % 2 == 0 else nc.gpsimd
                    eng.dma_start(out=x8_sb[g][:, c0:c0 + HW],
                                  in_=x8_in[g][:, c0:c0 + HW])
            load_bias("bqk", bqk_in, nc.sync)
            for g in range(G):
                t = wpool.tile([128, 2], fp8, tag=f"hbk8{g}", name=f"hbk8{g}")
                nc.gpsimd.dma_start(out=t[:], in_=hbk8_in[g][:])
                hbk8_sb.append(t)
            for pair in range(4):
                for g in range(G):
                    for i in range(2):
                        c0 = i * NCORES * HW + max(pair * 2 * HW, HW)
                        c1 = i * NCORES * HW + (pair + 1) * 2 * HW
                        if c1 <= c0:
                            continue
                        eng = nc.sync if (g + i + pair) % 2 == 0 else nc.gpsimd
                        eng.dma_start(out=x8_sb[g][:, c0:c1],
                                      in_=x8_in[g][:, c0:c1])
            # tail-path inputs
            x_sb = []
            for ci in range(CB):
                t = xpool.tile([128, HW], bf16, tag=f"x{ci}", name=f"x{ci}")
                nc.sync.dma_start(out=t[:],
                                  in_=x_in[ci * 128:(ci + 1) * 128, :])
                x_sb.append(t)
            w6_sb = []
            for ci in range(CB):
                t = wpool.tile([128, C], bf16, tag=f"w6{ci}", name=f"w6{ci}")
                nc.gpsimd.dma_start(out=t[:],
                                    in_=w6t_in[ci * 128:(ci + 1) * 128, :])
                w6_sb.append(t)
            load_bias("b6", b6_in, nc.gpsimd)

            ones_col = consts.tile([128, 1], f32, tag="ones_col")
            nc.vector.memset(ones_col[:], 1.0)
            ones_row = consts.tile([1, 128], f32, tag="ones_row")
            nc.vector.memset(ones_row[:], 1.0)
            ones_row_bf = consts.tile([1, 128], bf16, tag="ones_row_bf")
            nc.vector.memset(ones_row_bf[:], 1.0)
            # warm the ln+exp+identity table set so no ACT_TABLE_LOAD lands
            # mid-pipeline (containment check keeps it resident)
            warm = consts.tile([1, 1], f32, tag="warm")
            nc.scalar.activation(warm[:], ones_col[:1, :1], AF.Ln,
                                 bias=0.0, scale=1.0)
            nc.scalar.activation(warm[:], ones_col[:1, :1], AF.Exp,
                                 bias=0.0, scale=1.0)
            shift_col = consts.tile([128, 1], f32, tag="shift_col")
            nc.vector.memset(shift_col[:], -LSE_SHIFT)

            # ---- qt = Wqk @ x_own + bqk, fp8 DR, plane-paired over c ----
            qt = [qpool.tile([128, G * HW], fp8, tag=f"qt{g}", name=f"qt{g}")
                  for g in range(G)]
            for co in range(CB):
                g, i = co // 2, co % 2
                ps = psp.tile([128, 1024], f32, tag="ps", name="ps_qt")
                for h in range(KH):
                    for gg in range(G):
                        nc.tensor.matmul(
                            ps[:, h * 512:(h + 1) * 512],
                            dr3(wqk8_sb[gg][:, :], C)[:, :,
                                                      co * 128:(co + 1) * 128],
                            dr3(x8_sb[gg][:, :],
                                NCORES * HW)[:, :, h * 512:(h + 1) * 512],
                            start=(gg == 0), stop=(gg == G - 1), perf_mode=DR)
                nc.scalar.activation(
                    qt[g][:, i * HW:(i + 1) * HW], ps[:], AF.Identity,
                    bias=bias_sb["bqk"][:, co:co + 1], scale=1.0 / WQK_SCALE)

            def qt_ap(g, qb):
                return dr3(qt[g][:, :], HW)[:, :, qb * 128:(qb + 1) * 128]

            # per-(query,image) true max (DVE path), qb < NDVE
            mp = redpool.tile([128, NDVE * NIMG], f32, tag="mp", name="mp")
            # per-(query-block,image) shifted exp sums (LSE path)
            sacc = redpool.tile([128, (QB - NDVE) * NIMG], f32, tag="sacc",
                                name="sacc")
            # qbk partials accumulate here across the image loop
            ps_qbk = psp.tile([128, 1024], f32, tag="ps", name="ps_qbk")
            qbk_sb = redpool.tile([128, QB], f32, tag="qbk", name="qbk")

            # per-qb softmax state (filled incrementally during image 7)
            X8 = redpool.tile([128, QB], f32, tag="X8", name="X8")
            X8b = redpool.tile([128, QB], f32, tag="X8b", name="X8b")
            EX = redpool.tile([128, QB], f32, tag="EX", name="EX")
            S1t = redpool.tile([128, QB], f32, tag="S1t", name="S1t")
            lns = redpool.tile([128, (QB - NDVE) * NIMG], f32, tag="lns",
                               name="lns")
            wr_d = dram.tile([128, QB], f32, tag="wr_d", name="wr_d")
            wrow = redpool.tile([1, HW], f32, tag="wrow", name="wrow")
            bengs = (nc.sync, nc.gpsimd)

            def finalize_qb(qb):
                """After image 7's (qb) consumption: produce EX column and
                bounce it to the [1,1024] row, overlapped with the
                remaining query blocks' scores."""
                c = slice(qb, qb + 1)
                if qb < NDVE:
                    nc.vector.tensor_reduce(
                        X8[:, c], mp[:, qb * NIMG:(qb + 1) * NIMG],
                        axis=AX.X, op=ALU.add)
                else:
                    lq = slice((qb - NDVE) * NIMG, (qb - NDVE + 1) * NIMG)
                    nc.scalar.activation(lns[:, lq], sacc[:, lq], AF.Ln,
                                         bias=0.0, scale=1.0)
                    xl = redpool.tile([128, 1], f32, tag="xl", name="xl",
                                      bufs=3)
                    nc.vector.tensor_reduce(xl[:], lns[:, lq],
                                            axis=AX.X, op=ALU.add)
                    nc.vector.tensor_scalar(
                        X8[:, c], xl[:], scalar1=NIMG * LSE_SHIFT,
                        scalar2=1.0 / LSE_T, op0=ALU.add, op1=ALU.mult)
                nc.vector.tensor_add(X8b[:, c], X8[:, c], qbk_sb[:, c])
                nc.scalar.activation(EX[:, c], X8b[:, c], AF.Exp, bias=0.0,
                                     scale=SCALE / NIMG,
                                     accum_out=S1t[:, c])
                bengs[qb % 2].dma_start(out=wr_d[:, c], in_=EX[:, c])
                bengs[(qb + 1) % 2].dma_start(
                    out=wrow[0:1, qb * 128:(qb + 1) * 128],
                    in_=wr_d[:, c].transpose([1, 0]))

            # ---- image loop: 8 paired score tiles each ----
            # DVE-consumed and scalar-consumed query blocks alternate so
            # neither consumer bursts; image 7 interleaves the per-qb
            # softmax finalization.
            qb_order = [0, NDVE, 1, NDVE + 1, 2, NDVE + 2, 3, 4][:QB]
            for img in range(NIMG):
                # qbk partials: 1 query block per image (tiny, hides in
                # the loop); after image 7's, evacuate the full row.
                for gg in range(G):
                    nc.tensor.matmul(
                        ps_qbk[:, img:img + 1],
                        dr3(x8_sb[gg][:, :],
                            NCORES * HW)[:, :, img * 128:(img + 1) * 128],
                        dr3(hbk8_sb[gg][:, :], 1),
                        start=(gg == 0), stop=(gg == G - 1), perf_mode=DR)
                if img == NIMG - 1:
                    nc.scalar.activation(qbk_sb[:], ps_qbk[:, :QB],
                                         AF.Identity, bias=0.0,
                                         scale=float(NIMG) / HBK_SCALE)
                for qb in qb_order:
                    ps = psp.tile([128, 1024], f32, tag="ps", name="ps_sc")
                    for h in range(KH):
                        col0 = img * HW + h * 512
                        for gg in range(G):
                            nc.tensor.matmul(
                                ps[:, h * 512:(h + 1) * 512], qt_ap(gg, qb),
                                dr3(x8_sb[gg][:, :],
                                    NCORES * HW)[:, :, col0:col0 + 512],
                                start=(gg == 0), stop=(gg == G - 1),
                                perf_mode=DR)
                    if qb < NDVE:
                        col = qb * NIMG + img
                        nc.vector.tensor_reduce(
                            mp[:, col:col + 1], ps[:], axis=AX.X, op=ALU.max)
                    else:
                        scr = scrpool.tile([128, 1024], bf16, tag="scr",
                                           name="scr")
                        col = (qb - NDVE) * NIMG + img
                        nc.scalar.activation(
                            scr[:], ps[:], AF.Exp, bias=shift_col[:],
                            scale=LSE_T, accum_out=sacc[:, col:col + 1])
                    if img == NIMG - 1:
                        finalize_qb(qb)

            # ---- final conv on UNGATED x (bf16) during the img7 drain ----
            y_sb = [qpool.tile([128, HW], bf16, tag=f"y{co}", name=f"y{co}")
                    for co in range(CB)]
            for co in range(CB):
                ps = psp.tile([128, 1024], f32, tag="ps", name="ps_y")
                for h in range(KH):
                    sl = slice(h * 512, (h + 1) * 512)
                    for ci in range(CB):
                        nc.tensor.matmul(
                            ps[:, sl], w6_sb[ci][:, co * 128:(co + 1) * 128],
                            x_sb[ci][:, sl],
                            start=(ci == 0), stop=(ci == CB - 1))
                nc.scalar.activation(y_sb[co][:], ps[:], AF.Identity,
                                     bias=0.0, scale=1.0)

            # ---- total + reciprocal ----
            ps_tot = psp.tile([128, 1024], f32, tag="ps", name="ps_tot")
            nc.tensor.matmul(ps_tot[:1, :QB], ones_col[:], S1t[:],
                             start=True, stop=True)
            tot = redpool.tile([1, 1], f32, tag="tot", name="tot")
            nc.vector.tensor_reduce(tot[:], ps_tot[:1, :QB],
                                    axis=AX.X, op=ALU.add)
            rcp = redpool.tile([1, 1], f32, tag="rcp", name="rcp")
            nc.vector.reciprocal(rcp[:], tot[:])
            ps_rb = psp.tile([128, 1024], f32, tag="ps", name="ps_rb")
            nc.tensor.matmul(ps_rb[:, :1], ones_row[:], rcp[:],
                             start=True, stop=True)
            rb = redpool.tile([128, 1], f32, tag="rb", name="rb")
            nc.vector.tensor_copy(out=rb[:], in_=ps_rb[:, :1])

            wrow_bf = redpool.tile([1, HW], bf16, tag="wrow_bf",
                                   name="wrow_bf")
            nc.vector.tensor_copy(out=wrow_bf[:], in_=wrow[:])

            # broadcast to all partitions (bf16 matmul), fold 1/total into
            # the bf16 evacuation
            B_bf = redpool.tile([128, HW], bf16, tag="B_bf", name="B_bf")
            ps_b = psp.tile([128, 1024], f32, tag="ps", name="ps_b")
            for h in range(KH):
                nc.tensor.matmul(ps_b[:, h * 512:(h + 1) * 512],
                                 ones_row_bf[:],
                                 wrow_bf[0:1, h * 512:(h + 1) * 512],
                                 start=True, stop=True)
            nc.scalar.activation(B_bf[:, :], ps_b[:], AF.Identity, bias=0.0,
                                 scale=rb[:])

            # ---- gate y, add b6, write f32 out ----
            oengs = (nc.sync, nc.gpsimd)
            for co in range(CB):
                og = qpool.tile([128, HW], bf16, tag=f"og{co}",
                                name=f"og{co}")
                nc.vector.tensor_mul(og[:], y_sb[co][:], B_bf[:])
                o = outpool.tile([128, HW], f32, tag=f"o{co}", name=f"o{co}")
                nc.scalar.activation(o[:], og[:], AF.Identity,
                                     bias=bias_sb["b6"][:, co:co + 1],
                                     scale=1.0)
                oengs[co % 2].dma_start(
                    out=out_ext[co * 128:(co + 1) * 128, :], in_=o[:])

    nc.compile()
    return nc


_BUILT = {}


def _get_nc():
    if "nc" not in _BUILT:
        _BUILT["nc"] = build_kernel()
    return _BUILT["nc"]


def make_in_maps(x, Wq, bq, Wk, bk, W6, b6):
    import ml_dtypes
    e4 = ml_dtypes.float8_e4m3
    bfl = ml_dtypes.bfloat16
    x = np.asarray(x, dtype=np.float32).reshape(B, C, HW)
    Wq = np.asarray(Wq, np.float32)
    Wk = np.asarray(Wk, np.float32)
    bq = np.asarray(bq, np.float32)
    bk = np.asarray(bk, np.float32)
    w6t = np.ascontiguousarray(np.asarray(W6, np.float32).T).astype(bfl)
    b6c = np.ascontiguousarray(np.asarray(b6, np.float32).reshape(C, 1))

    # host-folded score factorization
    Wqk = Wk.T @ Wq                       # [c_tilde, c_in]
    bqk = (Wk.T @ bq).reshape(C, 1)
    hbk = (Wq.T @ bk).reshape(C, 1)

    def dr_pack(ws):
        """[c(contraction), M] fp32 -> G x [128, 2*M] fp8 plane-paired."""
        M = ws.shape[1]
        w8 = ws.astype(e4).reshape(G, 2, 128, M)
        return [np.ascontiguousarray(
            np.transpose(w8[g], (1, 0, 2)).reshape(128, 2 * M))
            for g in range(G)]

    wqk8g = dr_pack(Wqk.T * WQK_SCALE)    # lhsT layout [c_in, c_tilde]
    hbk8g = dr_pack(hbk * HBK_SCALE)      # [c_in, 1]
    # fp8 DoubleRow layouts of x for every core, image-rolled so slot 0 is
    # the core's own image: c = g*256 + i*128 + p
    xc = np.transpose(x, (1, 0, 2))                      # [c, img, hw]
    x8_f = xc.astype(e4)
    maps = []
    for b in range(B):
        order = [(b + i) % B for i in range(B)]
        xr = x8_f[:, order, :].reshape(G, 2, 128, B * HW)
        x8g = [np.ascontiguousarray(
            np.transpose(xr[g], (1, 0, 2)).reshape(128, 2 * B * HW))
            for g in range(G)]
        m = {"x": np.ascontiguousarray(x[b]).astype(bfl),
             "w6t": w6t, "bqk": bqk, "b6": b6c}
        for g in range(G):
            m[f"x8g{g}"] = x8g[g]
            m[f"wqk8g{g}"] = wqk8g[g]
            m[f"hbk8g{g}"] = hbk8g[g]
        maps.append(m)
    return maps


def kernel(x, Wq, bq, Wk, bk, W6, b6, _trace=False):
    from concourse import bass_utils
    nc = _get_nc()
    in_maps = make_in_maps(x, Wq, bq, Wk, bk, W6, b6)
    res = bass_utils.run_bass_kernel_spmd(
        nc, in_maps, core_ids=list(range(NCORES)), trace=_trace)
    out = np.stack([np.asarray(res.results[i]["out"]) for i in range(NCORES)])
    out = out.reshape(B, C, H, W).astype(np.float32)
    if _trace:
        return out, res
    return out


# revision 28
# speedup vs baseline: 1.1116x; 1.1116x over previous
"""Trainium2 Bass kernel for nn_AllAttLayer (cross-batch attention gating layer).

Reference computation (B=8, C=512, H=W=32, HW=1024):
    xf = x as [B, HW, C]
    q = xf @ Wq.T + bq ; k = xf @ Wk.T + bk
    scores = q.flat @ k.flat.T                  # [B*HW, B*HW]
    xw = max over each image's keys, mean over images   # [B*HW]
    xw = softmax(xw * C**-0.5 per image)        # [B, HW]
    out = (x * xw) @ W6.T + b6  (1x1 conv)      # == W6 @ (x * xw)

Key algebraic restructure: scores = q^T k = q^T (Wk x + bk)
  = (Wk^T Wq x + Wk^T bq)^T x  +  (x^T Wq^T bk + bq^T bk).
The host folds Wqk := Wk^T Wq and bqk := Wk^T bq, so the kernel projects
each core's queries ONCE (qt = Wqk x + bqk, fp8 DoubleRow from the
replicated fp8 x) and the score matmuls consume the replicated fp8 x
DIRECTLY as the moving operand -- no per-image key projection, no key
evacuations. The k-bias term x^T(Wq^T bk) =: qbk is a 1-column fp8
projection (host-folded hbk := Wq^T bk) added to the logits; its
constant part bq^T bk is uniform over all queries and cancels in
softmax.

Sharding: core b owns image b (its 1024 queries). No collectives: the
host replicates x in fp8 DoubleRow layout, ROLLED per core so the
core's own image is slot 0 (the kernel is SPMD -- same program, per-core
data). Everything is c-major ([C, HW]) so no transposes are needed.

Engine schedule: per (query-block, image) the two 512-key score halves
land in one paired [128,1024] PSUM tile (2 banks); query blocks 0..NDVE-1
are consumed by a single DVE max-reduce (~1.19us), the rest by a
ScalarE exp-accumulate (LSE max approximation with temperature 2 and a
-80 shift to keep exp sums in range; the ~ln(n_eff)/2 overestimate is
~0.5 on logits*SCALE/8 ~ 0.003, well under the tolerance). Per image:
PE 7.6us of score matmuls vs DVE ~6.0us + scalar ~3.5us -- PE-paced.
The final conv runs bf16 on ungated x DURING the last image's score
drain; the gate (and b6) applies at the tail. fp32 elsewhere.
"""

import sys
import numpy as np

for _p in ("/opt/trn_rl_repo",):
    if _p not in sys.path:
        sys.path.insert(0, _p)

B, C, H, W = 8, 512, 32, 32
HW = H * W              # 1024 pixels per image
NCORES = 8
CB = C // 128           # 4 channel blocks
G = 2                   # DoubleRow groups (K=256 each)
QB = HW // 128          # 8 query blocks per core
KH = 2                  # key halves (512 keys each)
NIMG = NCORES
SCALE = 1.0 / float(np.sqrt(C))

WQK_SCALE = 64.0        # host scales Wqk by this before fp8
HBK_SCALE = 16.0        # host scales hbk by this before fp8
NDVE = 5                # query blocks per image consumed by DVE (rest: LSE)
LSE_T = 2.0             # LSE temperature
LSE_SHIFT = 36.0        # exp(t*s-SHIFT): sums stay in the HW ln
                        # spline's valid range [1e-18, 1e19]


def build_kernel():
    from concourse import bacc, tile, mybir

    f32 = mybir.dt.float32
    bf16 = mybir.dt.bfloat16
    fp8 = mybir.dt.float8e4
    DR = mybir.MatmulPerfMode.DoubleRow

    nc = bacc.Bacc("TRN2", target_bir_lowering=False, debug=False,
                   num_devices=NCORES)

    x_in = nc.dram_tensor("x", [C, HW], bf16, kind="ExternalInput").ap()
    w6t_in = nc.dram_tensor("w6t", [C, C], bf16, kind="ExternalInput").ap()
    x8_in = [nc.dram_tensor(f"x8g{g}", [128, 2 * NCORES * HW], fp8,
                            kind="ExternalInput").ap() for g in range(G)]
    wqk8_in = [nc.dram_tensor(f"wqk8g{g}", [128, 2 * C], fp8,
                              kind="ExternalInput").ap() for g in range(G)]
    hbk8_in = [nc.dram_tensor(f"hbk8g{g}", [128, 2], fp8,
                              kind="ExternalInput").ap() for g in range(G)]
    bqk_in = nc.dram_tensor("bqk", [C, 1], f32, kind="ExternalInput").ap()
    idm_in = nc.dram_tensor("idm", [128, 128], f32, kind="ExternalInput").ap()
    selm_in = nc.dram_tensor("selm", [QB, QB * 128], bf16,
                             kind="ExternalInput").ap()
    b6_in = nc.dram_tensor("b6", [C, 1], f32, kind="ExternalInput").ap()
    out_ext = nc.dram_tensor("out", [C, HW], f32, kind="ExternalOutput").ap()

    AF = mybir.ActivationFunctionType
    ALU = mybir.AluOpType
    AX = mybir.AxisListType

    def dr3(ap, span):
        """[128, G*span] tile AP -> [128, 2, span] DoubleRow view."""
        return ap.rearrange("p (i n) -> p i n", i=2, n=span)

    with tile.TileContext(nc) as tc:
        with tc.tile_pool(name="consts", bufs=1) as consts, \
             tc.tile_pool(name="wpool", bufs=1) as wpool, \
             tc.tile_pool(name="xpool", bufs=1) as xpool, \
             tc.tile_pool(name="qpool", bufs=1) as qpool, \
             tc.tile_pool(name="redpool", bufs=1) as redpool, \
             tc.tile_pool(name="scrpool", bufs=3) as scrpool, \
             tc.tile_pool(name="outpool", bufs=1) as outpool, \
             tc.tile_pool(name="dram", bufs=1, space="DRAM") as dram, \
             tc.tile_pool(name="ps", bufs=4, space="PSUM") as psp:

            bias_sb = {}

            def load_bias(nm, src, eng):
                t = consts.tile([128, CB], f32, tag=f"{nm}_sb", name=f"{nm}_sb")
                for co in range(CB):
                    eng.dma_start(out=t[:, co:co + 1],
                                  in_=src[co * 128:(co + 1) * 128, :])
                bias_sb[nm] = t

            # ---- head loads ----
            # own-image x8 slices (slot 0) and Wqk first (they gate the
            # qt projection); the scalar queue gets no head DMAs.
            x8_sb = []
            for g in range(G):
                t = xpool.tile([128, 2 * NCORES * HW], fp8, tag=f"x8{g}",
                               name=f"x8{g}")
                x8_sb.append(t)
            wqk8_sb, hbk8_sb = [], []
            for g in range(G):
                t = wpool.tile([128, 2 * C], fp8, tag=f"wqk8{g}",
                               name=f"wqk8{g}")
                nc.sync.dma_start(out=t[:], in_=wqk8_in[g][:])
                wqk8_sb.append(t)
            for g in range(G):
                for i in range(2):
                    c0 = i * NCORES * HW
                    eng = nc.sync if (g + i) % 2 == 0 else nc.gpsimd
                    eng.dma_start(out=x8_sb[g][:, c0:c0 + HW],
                                  in_=x8_in[g][:, c0:c0 + HW])
            load_bias("bqk", bqk_in, nc.sync)
            for g in range(G):
                t = wpool.tile([128, 2], fp8, tag=f"hbk8{g}", name=f"hbk8{g}")
                nc.gpsimd.dma_start(out=t[:], in_=hbk8_in[g][:])
                hbk8_sb.append(t)
            for pair in range(4):
                for g in range(G):
                    for i in range(2):
                        c0 = i * NCORES * HW + max(pair * 2 * HW, HW)
                        c1 = i * NCORES * HW + (pair + 1) * 2 * HW
                        if c1 <= c0:
                            continue
                        eng = nc.sync if (g + i + pair) % 2 == 0 else nc.gpsimd
                        eng.dma_start(out=x8_sb[g][:, c0:c1],
                                      in_=x8_in[g][:, c0:c1])
            # tail-path inputs
            x_sb = []
            for ci in range(CB):
                t = xpool.tile([128, HW], bf16, tag=f"x{ci}", name=f"x{ci}")
                nc.sync.dma_start(out=t[:],
                                  in_=x_in[ci * 128:(ci + 1) * 128, :])
                x_sb.append(t)
            w6_sb = []
            for ci in range(CB):
                t = wpool.tile([128, C], bf16, tag=f"w6{ci}", name=f"w6{ci}")
                nc.gpsimd.dma_start(out=t[:],
                                    in_=w6t_in[ci * 128:(ci + 1) * 128, :])
                w6_sb.append(t)
            load_bias("b6", b6_in, nc.gpsimd)
            idm_sb = wpool.tile([128, 128], f32, tag="idm", name="idm")
            nc.sync.dma_start(out=idm_sb[:], in_=idm_in[:])
            selm_sb = wpool.tile([QB, QB * 128], bf16, tag="selm",
                                 name="selm")
            nc.gpsimd.dma_start(out=selm_sb[:], in_=selm_in[:])

            ones_col = consts.tile([128, 1], f32, tag="ones_col")
            nc.vector.memset(ones_col[:], 1.0)
            ones_row = consts.tile([1, 128], f32, tag="ones_row")
            nc.vector.memset(ones_row[:], 1.0)
            # warm the ln+exp+identity table set so no ACT_TABLE_LOAD lands
            # mid-pipeline (containment check keeps it resident)
            warm = consts.tile([1, 1], f32, tag="warm")
            nc.scalar.activation(warm[:], ones_col[:1, :1], AF.Ln,
                                 bias=0.0, scale=1.0)
            nc.scalar.activation(warm[:], ones_col[:1, :1], AF.Exp,
                                 bias=0.0, scale=1.0)
            shift_col = consts.tile([128, 1], f32, tag="shift_col")
            nc.vector.memset(shift_col[:], -LSE_SHIFT)

            # ---- qt = Wqk @ x_own + bqk, fp8 DR, plane-paired over c ----
            qt = [qpool.tile([128, G * HW], fp8, tag=f"qt{g}", name=f"qt{g}")
                  for g in range(G)]
            for co in range(CB):
                g, i = co // 2, co % 2
                ps = psp.tile([128, 1024], f32, tag="ps", name="ps_qt")
                for h in range(KH):
                    for gg in range(G):
                        nc.tensor.matmul(
                            ps[:, h * 512:(h + 1) * 512],
                            dr3(wqk8_sb[gg][:, :], C)[:, :,
                                                      co * 128:(co + 1) * 128],
                            dr3(x8_sb[gg][:, :],
                                NCORES * HW)[:, :, h * 512:(h + 1) * 512],
                            start=(gg == 0), stop=(gg == G - 1), perf_mode=DR)
                nc.scalar.activation(
                    qt[g][:, i * HW:(i + 1) * HW], ps[:], AF.Identity,
                    bias=bias_sb["bqk"][:, co:co + 1], scale=1.0 / WQK_SCALE)

            def qt_ap(g, qb):
                return dr3(qt[g][:, :], HW)[:, :, qb * 128:(qb + 1) * 128]

            # per-(query,image) true max (DVE path), qb < NDVE
            mp = redpool.tile([128, NDVE * NIMG], f32, tag="mp", name="mp")
            # per-(query-block,image) shifted exp sums (LSE path)
            sacc = redpool.tile([128, (QB - NDVE) * NIMG], f32, tag="sacc",
                                name="sacc")
            # qbk partials accumulate here across the image loop
            ps_qbk = psp.tile([128, 1024], f32, tag="ps", name="ps_qbk")
            qbk_sb = redpool.tile([128, QB], f32, tag="qbk", name="qbk")

            # per-qb softmax state (filled incrementally during image 7)
            X8 = redpool.tile([128, QB], f32, tag="X8", name="X8")
            X8b = redpool.tile([128, QB], f32, tag="X8b", name="X8b")
            EX = redpool.tile([128, QB], f32, tag="EX", name="EX")
            S1t = redpool.tile([128, QB], f32, tag="S1t", name="S1t")
            lns = redpool.tile([128, (QB - NDVE) * NIMG], f32, tag="lns",
                               name="lns")

            def ex_col(qb):
                c = slice(qb, qb + 1)
                nc.vector.tensor_add(X8b[:, c], X8[:, c], qbk_sb[:, c])
                nc.scalar.activation(EX[:, c], X8b[:, c], AF.Exp, bias=0.0,
                                     scale=SCALE / NIMG,
                                     accum_out=S1t[:, c])

            def finalize_lse():
                """One batched ln over all LSE accumulators (a single Ln
                keeps the activation-table thrash to one round trip),
                emitted early in image 7 so the table loads overlap the
                remaining score matmuls."""
                nq = QB - NDVE
                nc.scalar.activation(lns[:], sacc[:], AF.Ln, bias=0.0,
                                     scale=1.0)
                xl = redpool.tile([128, nq], f32, tag="xl", name="xl")
                nc.vector.tensor_reduce(
                    xl[:],
                    lns[:, :].rearrange("p (q i) -> p q i", q=nq, i=NIMG),
                    axis=AX.X, op=ALU.add)
                nc.vector.tensor_scalar(
                    X8[:, NDVE:], xl[:], scalar1=NIMG * LSE_SHIFT,
                    scalar2=1.0 / LSE_T, op0=ALU.add, op1=ALU.mult)
                for qb in range(NDVE, QB):
                    ex_col(qb)

            def finalize_dve(qb):
                c = slice(qb, qb + 1)
                nc.vector.tensor_reduce(
                    X8[:, c], mp[:, qb * NIMG:(qb + 1) * NIMG],
                    axis=AX.X, op=ALU.add)
                ex_col(qb)

            # ---- image loop: 8 paired score tiles each ----
            # LSE query blocks go first so image 7 can run its single
            # batched ln while the DVE blocks' scores still stream.
            qb_order = list(range(NDVE, QB)) + list(range(NDVE))
            for img in range(NIMG):
                # qbk partials: 1 query block per image (tiny, hides in
                # the loop); after image 7's, evacuate the full row.
                for gg in range(G):
                    nc.tensor.matmul(
                        ps_qbk[:, img:img + 1],
                        dr3(x8_sb[gg][:, :],
                            NCORES * HW)[:, :, img * 128:(img + 1) * 128],
                        dr3(hbk8_sb[gg][:, :], 1),
                        start=(gg == 0), stop=(gg == G - 1), perf_mode=DR)
                if img == NIMG - 1:
                    nc.scalar.activation(qbk_sb[:], ps_qbk[:, :QB],
                                         AF.Identity, bias=0.0,
                                         scale=float(NIMG) / HBK_SCALE)
                for pos, qb in enumerate(qb_order):
                    ps = psp.tile([128, 1024], f32, tag="ps", name="ps_sc")
                    for h in range(KH):
                        col0 = img * HW + h * 512
                        for gg in range(G):
                            nc.tensor.matmul(
                                ps[:, h * 512:(h + 1) * 512], qt_ap(gg, qb),
                                dr3(x8_sb[gg][:, :],
                                    NCORES * HW)[:, :, col0:col0 + 512],
                                start=(gg == 0), stop=(gg == G - 1),
                                perf_mode=DR)
                    if qb < NDVE:
                        col = qb * NIMG + img
                        nc.vector.tensor_reduce(
                            mp[:, col:col + 1], ps[:], axis=AX.X, op=ALU.max)
                    else:
                        scr = scrpool.tile([128, 1024], bf16, tag="scr",
                                           name="scr")
                        col = (qb - NDVE) * NIMG + img
                        nc.scalar.activation(
                            scr[:], ps[:], AF.Exp, bias=shift_col[:],
                            scale=LSE_T, accum_out=sacc[:, col:col + 1])
                    if img == NIMG - 1:
                        if pos == QB - NDVE - 1:
                            finalize_lse()
                        elif qb < NDVE:
                            finalize_dve(qb)

            # ---- final conv on UNGATED x (bf16) during the img7 drain ----
            y_sb = [qpool.tile([128, HW], bf16, tag=f"y{co}", name=f"y{co}")
                    for co in range(CB)]
            for co in range(CB):
                ps = psp.tile([128, 1024], f32, tag="ps", name="ps_y")
                for h in range(KH):
                    sl = slice(h * 512, (h + 1) * 512)
                    for ci in range(CB):
                        nc.tensor.matmul(
                            ps[:, sl], w6_sb[ci][:, co * 128:(co + 1) * 128],
                            x_sb[ci][:, sl],
                            start=(ci == 0), stop=(ci == CB - 1))
                nc.scalar.activation(y_sb[co][:], ps[:], AF.Identity,
                                     bias=0.0, scale=1.0)

            # ---- total + reciprocal ----
            ps_tot = psp.tile([128, 1024], f32, tag="ps", name="ps_tot")
            nc.tensor.matmul(ps_tot[:1, :QB], ones_col[:], S1t[:],
                             start=True, stop=True)
            tot = redpool.tile([1, 1], f32, tag="tot", name="tot")
            nc.vector.tensor_reduce(tot[:], ps_tot[:1, :QB],
                                    axis=AX.X, op=ALU.add)
            rcp = redpool.tile([1, 1], f32, tag="rcp", name="rcp")
            nc.vector.reciprocal(rcp[:], tot[:])

            # ---- broadcast EX[p, qb] -> B[:, qb*128+p] on the PE ----
            # transpose EX via an identity matmul, then 8 one-hot selector
            # matmuls replicate row qb across all partitions of block qb.
            # No DRAM bounce, no per-column DMAs.
            ps_t = psp.tile([128, 1024], f32, tag="ps", name="ps_t")
            nc.tensor.matmul(ps_t[:QB, :128], EX[:, :], idm_sb[:],
                             start=True, stop=True)
            ext_bf = redpool.tile([QB, 128], bf16, tag="ext_bf",
                                  name="ext_bf")
            nc.scalar.activation(ext_bf[:], ps_t[:QB, :128], AF.Identity,
                                 bias=0.0, scale=1.0)
            ps_b = psp.tile([128, 1024], f32, tag="ps", name="ps_b")
            for qb in range(QB):
                nc.tensor.matmul(ps_b[:, qb * 128:(qb + 1) * 128],
                                 selm_sb[:, qb * 128:(qb + 1) * 128],
                                 ext_bf[:], start=True, stop=True)
            ps_rb = psp.tile([128, 1024], f32, tag="ps", name="ps_rb")
            nc.tensor.matmul(ps_rb[:, :1], ones_row[:], rcp[:],
                             start=True, stop=True)
            rb = redpool.tile([128, 1], f32, tag="rb", name="rb")
            nc.vector.tensor_copy(out=rb[:], in_=ps_rb[:, :1])
            B_bf = redpool.tile([128, HW], bf16, tag="B_bf", name="B_bf")
            nc.scalar.activation(B_bf[:, :], ps_b[:], AF.Identity, bias=0.0,
                                 scale=rb[:])

            # ---- gate y, add b6, write f32 out ----
            oengs = (nc.sync, nc.gpsimd)
            for co in range(CB):
                og = qpool.tile([128, HW], bf16, tag=f"og{co}",
                                name=f"og{co}")
                nc.vector.tensor_mul(og[:], y_sb[co][:], B_bf[:])
                o = outpool.tile([128, HW], f32, tag=f"o{co}", name=f"o{co}")
                nc.scalar.activation(o[:], og[:], AF.Identity,
                                     bias=bias_sb["b6"][:, co:co + 1],
                                     scale=1.0)
                oengs[co % 2].dma_start(
                    out=out_ext[co * 128:(co + 1) * 128, :], in_=o[:])

    nc.compile()
    return nc


_BUILT = {}


def _get_nc():
    if "nc" not in _BUILT:
        _BUILT["nc"] = build_kernel()
    return _BUILT["nc"]


def make_in_maps(x, Wq, bq, Wk, bk, W6, b6):
    import ml_dtypes
    e4 = ml_dtypes.float8_e4m3
    bfl = ml_dtypes.bfloat16
    x = np.asarray(x, dtype=np.float32).reshape(B, C, HW)
    Wq = np.asarray(Wq, np.float32)
    Wk = np.asarray(Wk, np.float32)
    bq = np.asarray(bq, np.float32)
    bk = np.asarray(bk, np.float32)
    w6t = np.ascontiguousarray(np.asarray(W6, np.float32).T).astype(bfl)
    b6c = np.ascontiguousarray(np.asarray(b6, np.float32).reshape(C, 1))

    # host-folded score factorization
    Wqk = Wk.T @ Wq                       # [c_tilde, c_in]
    bqk = (Wk.T @ bq).reshape(C, 1)
    hbk = (Wq.T @ bk).reshape(C, 1)

    def dr_pack(ws):
        """[c(contraction), M] fp32 -> G x [128, 2*M] fp8 plane-paired."""
        M = ws.shape[1]
        w8 = ws.astype(e4).reshape(G, 2, 128, M)
        return [np.ascontiguousarray(
            np.transpose(w8[g], (1, 0, 2)).reshape(128, 2 * M))
            for g in range(G)]

    wqk8g = dr_pack(Wqk.T * WQK_SCALE)    # lhsT layout [c_in, c_tilde]
    hbk8g = dr_pack(hbk * HBK_SCALE)      # [c_in, 1]
    idm_h = np.eye(128, dtype=np.float32)
    selm_h = np.zeros((QB, QB * 128), dtype=bfl)
    for qb in range(QB):
        selm_h[qb, qb * 128:(qb + 1) * 128] = 1
    # fp8 DoubleRow layouts of x for every core, image-rolled so slot 0 is
    # the core's own image: c = g*256 + i*128 + p
    xc = np.transpose(x, (1, 0, 2))                      # [c, img, hw]
    x8_f = xc.astype(e4)
    maps = []
    for b in range(B):
        order = [(b + i) % B for i in range(B)]
        xr = x8_f[:, order, :].reshape(G, 2, 128, B * HW)
        x8g = [np.ascontiguousarray(
            np.transpose(xr[g], (1, 0, 2)).reshape(128, 2 * B * HW))
            for g in range(G)]
        m = {"x": np.ascontiguousarray(x[b]).astype(bfl),
             "w6t": w6t, "bqk": bqk, "b6": b6c, "idm": idm_h,
             "selm": selm_h}
        for g in range(G):
            m[f"x8g{g}"] = x8g[g]
            m[f"wqk8g{g}"] = wqk8g[g]
            m[f"hbk8g{g}"] = hbk8g[g]
        maps.append(m)
    return maps


def kernel(x, Wq, bq, Wk, bk, W6, b6, _trace=False):
    from concourse import bass_utils
    nc = _get_nc()
    in_maps = make_in_maps(x, Wq, bq, Wk, bk, W6, b6)
    res = bass_utils.run_bass_kernel_spmd(
        nc, in_maps, core_ids=list(range(NCORES)), trace=_trace)
    out = np.stack([np.asarray(res.results[i]["out"]) for i in range(NCORES)])
    out = out.reshape(B, C, H, W).astype(np.float32)
    if _trace:
        return out, res
    return out


# revision 31
# speedup vs baseline: 1.1973x; 1.0772x over previous
"""Trainium2 Bass kernel for nn_AllAttLayer (cross-batch attention gating layer).

Reference computation (B=8, C=512, H=W=32, HW=1024):
    xf = x as [B, HW, C]
    q = xf @ Wq.T + bq ; k = xf @ Wk.T + bk
    scores = q.flat @ k.flat.T                  # [B*HW, B*HW]
    xw = max over each image's keys, mean over images   # [B*HW]
    xw = softmax(xw * C**-0.5 per image)        # [B, HW]
    out = (x * xw) @ W6.T + b6  (1x1 conv)      # == W6 @ (x * xw)

Key algebraic restructure: scores = q^T k = q^T (Wk x + bk)
  = (Wk^T Wq x + Wk^T bq)^T x  +  (x^T Wq^T bk + bq^T bk).
The host folds Wqk := Wk^T Wq and bqk := Wk^T bq, so the kernel projects
each core's queries ONCE (qt = Wqk x + bqk, fp8 DoubleRow from the
replicated fp8 x) and the score matmuls consume the replicated fp8 x
DIRECTLY as the moving operand -- no per-image key projection, no key
evacuations. The k-bias term x^T(Wq^T bk) =: qbk is a 1-column fp8
projection (host-folded hbk := Wq^T bk) added to the logits; its
constant part bq^T bk is uniform over all queries and cancels in
softmax.

Sharding: core b owns image b (its 1024 queries). No collectives: the
host replicates x in fp8 DoubleRow layout, ROLLED per core so the
core's own image is slot 0 (the kernel is SPMD -- same program, per-core
data). Everything is c-major ([C, HW]) so no transposes are needed.

Engine schedule: per (query-block, image) the two 512-key score halves
land in one paired [128,1024] PSUM tile (2 banks); query blocks 0..NDVE-1
are consumed by a single DVE max-reduce (~1.19us), the rest by a
ScalarE exp-accumulate (LSE max approximation with temperature 2 and a
-80 shift to keep exp sums in range; the ~ln(n_eff)/2 overestimate is
~0.5 on logits*SCALE/8 ~ 0.003, well under the tolerance). Per image:
PE 7.6us of score matmuls vs DVE ~6.0us + scalar ~3.5us -- PE-paced.
The final conv runs bf16 on ungated x DURING the last image's score
drain; the gate (and b6) applies at the tail. fp32 elsewhere.
"""

import sys
import numpy as np

for _p in ("/opt/trn_rl_repo",):
    if _p not in sys.path:
        sys.path.insert(0, _p)

B, C, H, W = 8, 512, 32, 32
HW = H * W              # 1024 pixels per image
NCORES = 8
CB = C // 128           # 4 channel blocks
G = 2                   # DoubleRow groups (K=256 each)
QB = HW // 128          # 8 query blocks per core
KH = 2                  # key halves (512 keys each)
NIMG = NCORES
SCALE = 1.0 / float(np.sqrt(C))

WQK_SCALE = 64.0        # host scales Wqk by this before fp8
HBK_SCALE = 16.0        # host scales hbk by this before fp8
NDVE = 4                # query blocks per image consumed by DVE (rest: LSE)
LSE_T = 2.0             # LSE temperature
LSE_SHIFT = 36.0        # exp(t*s-SHIFT): sums stay in the HW ln
                        # spline's valid range [1e-18, 1e19]


def build_kernel():
    from concourse import bacc, tile, mybir

    f32 = mybir.dt.float32
    bf16 = mybir.dt.bfloat16
    fp8 = mybir.dt.float8e4
    DR = mybir.MatmulPerfMode.DoubleRow

    nc = bacc.Bacc("TRN2", target_bir_lowering=False, debug=False,
                   num_devices=NCORES)

    x_in = nc.dram_tensor("x", [C, HW], bf16, kind="ExternalInput").ap()
    w6t_in = nc.dram_tensor("w6t", [C, C], bf16, kind="ExternalInput").ap()
    x8_in = [nc.dram_tensor(f"x8g{g}", [128, 2 * NCORES * HW], fp8,
                            kind="ExternalInput").ap() for g in range(G)]
    wqk8_in = [nc.dram_tensor(f"wqk8g{g}", [128, 2 * C], fp8,
                              kind="ExternalInput").ap() for g in range(G)]
    hbk8_in = [nc.dram_tensor(f"hbk8g{g}", [128, 2], fp8,
                              kind="ExternalInput").ap() for g in range(G)]
    bqk_in = nc.dram_tensor("bqk", [C, 1], f32, kind="ExternalInput").ap()
    idm_in = nc.dram_tensor("idm", [128, 128], f32, kind="ExternalInput").ap()
    selm_in = nc.dram_tensor("selm", [QB, QB * 128], bf16,
                             kind="ExternalInput").ap()
    b6_in = nc.dram_tensor("b6", [C, 1], f32, kind="ExternalInput").ap()
    out_ext = nc.dram_tensor("out", [C, HW], bf16,
                             kind="ExternalOutput").ap()

    AF = mybir.ActivationFunctionType
    ALU = mybir.AluOpType
    AX = mybir.AxisListType

    def dr3(ap, span):
        """[128, G*span] tile AP -> [128, 2, span] DoubleRow view."""
        return ap.rearrange("p (i n) -> p i n", i=2, n=span)

    with tile.TileContext(nc) as tc:
        with tc.tile_pool(name="consts", bufs=1) as consts, \
             tc.tile_pool(name="wpool", bufs=1) as wpool, \
             tc.tile_pool(name="xpool", bufs=1) as xpool, \
             tc.tile_pool(name="qpool", bufs=1) as qpool, \
             tc.tile_pool(name="redpool", bufs=1) as redpool, \
             tc.tile_pool(name="scrpool", bufs=3) as scrpool, \
             tc.tile_pool(name="outpool", bufs=1) as outpool, \
             tc.tile_pool(name="dram", bufs=1, space="DRAM") as dram, \
             tc.tile_pool(name="ps", bufs=4, space="PSUM") as psp:

            bias_sb = {}

            def load_bias(nm, src, eng):
                t = consts.tile([128, CB], f32, tag=f"{nm}_sb", name=f"{nm}_sb")
                for co in range(CB):
                    eng.dma_start(out=t[:, co:co + 1],
                                  in_=src[co * 128:(co + 1) * 128, :])
                bias_sb[nm] = t

            # ---- head loads ----
            # own-image x8 slices (slot 0) and Wqk first (they gate the
            # qt projection); the scalar queue gets no head DMAs.
            x8_sb = []
            for g in range(G):
                t = xpool.tile([128, 2 * NCORES * HW], fp8, tag=f"x8{g}",
                               name=f"x8{g}")
                x8_sb.append(t)
            wqk8_sb, hbk8_sb = [], []
            for g in range(G):
                t = wpool.tile([128, 2 * C], fp8, tag=f"wqk8{g}",
                               name=f"wqk8{g}")
                nc.sync.dma_start(out=t[:], in_=wqk8_in[g][:])
                wqk8_sb.append(t)
            for g in range(G):
                for i in range(2):
                    c0 = i * NCORES * HW
                    eng = nc.sync if (g + i) % 2 == 0 else nc.gpsimd
                    eng.dma_start(out=x8_sb[g][:, c0:c0 + HW],
                                  in_=x8_in[g][:, c0:c0 + HW])
            load_bias("bqk", bqk_in, nc.sync)
            for g in range(G):
                t = wpool.tile([128, 2], fp8, tag=f"hbk8{g}", name=f"hbk8{g}")
                nc.gpsimd.dma_start(out=t[:], in_=hbk8_in[g][:])
                hbk8_sb.append(t)
            for pair in range(4):
                for g in range(G):
                    for i in range(2):
                        c0 = i * NCORES * HW + max(pair * 2 * HW, HW)
                        c1 = i * NCORES * HW + (pair + 1) * 2 * HW
                        if c1 <= c0:
                            continue
                        eng = nc.sync if (g + i + pair) % 2 == 0 else nc.gpsimd
                        eng.dma_start(out=x8_sb[g][:, c0:c1],
                                      in_=x8_in[g][:, c0:c1])
            # tail-path inputs
            x_sb = []
            for ci in range(CB):
                t = xpool.tile([128, HW], bf16, tag=f"x{ci}", name=f"x{ci}")
                nc.sync.dma_start(out=t[:],
                                  in_=x_in[ci * 128:(ci + 1) * 128, :])
                x_sb.append(t)
            w6_sb = []
            for ci in range(CB):
                t = wpool.tile([128, C], bf16, tag=f"w6{ci}", name=f"w6{ci}")
                nc.gpsimd.dma_start(out=t[:],
                                    in_=w6t_in[ci * 128:(ci + 1) * 128, :])
                w6_sb.append(t)
            load_bias("b6", b6_in, nc.gpsimd)
            idm_sb = wpool.tile([128, 128], f32, tag="idm", name="idm")
            nc.sync.dma_start(out=idm_sb[:], in_=idm_in[:])
            selm_sb = wpool.tile([QB, QB * 128], bf16, tag="selm",
                                 name="selm")
            nc.gpsimd.dma_start(out=selm_sb[:], in_=selm_in[:])

            ones_col = consts.tile([128, 1], f32, tag="ones_col")
            nc.vector.memset(ones_col[:], 1.0)
            ones_row = consts.tile([1, 128], f32, tag="ones_row")
            nc.vector.memset(ones_row[:], 1.0)
            # warm the ln+exp+identity table set so no ACT_TABLE_LOAD lands
            # mid-pipeline (containment check keeps it resident)
            warm = consts.tile([1, 1], f32, tag="warm")
            nc.scalar.activation(warm[:], ones_col[:1, :1], AF.Ln,
                                 bias=0.0, scale=1.0)
            nc.scalar.activation(warm[:], ones_col[:1, :1], AF.Exp,
                                 bias=0.0, scale=1.0)
            shift_col = consts.tile([128, 1], f32, tag="shift_col")
            nc.vector.memset(shift_col[:], -LSE_SHIFT)

            # ---- qt = Wqk @ x_own + bqk, fp8 DR, plane-paired over c ----
            qt = [qpool.tile([128, G * HW], fp8, tag=f"qt{g}", name=f"qt{g}")
                  for g in range(G)]
            for co in range(CB):
                g, i = co // 2, co % 2
                ps = psp.tile([128, 1024], f32, tag="ps", name="ps_qt")
                for h in range(KH):
                    for gg in range(G):
                        nc.tensor.matmul(
                            ps[:, h * 512:(h + 1) * 512],
                            dr3(wqk8_sb[gg][:, :], C)[:, :,
                                                      co * 128:(co + 1) * 128],
                            dr3(x8_sb[gg][:, :],
                                NCORES * HW)[:, :, h * 512:(h + 1) * 512],
                            start=(gg == 0), stop=(gg == G - 1), perf_mode=DR)
                nc.scalar.activation(
                    qt[g][:, i * HW:(i + 1) * HW], ps[:], AF.Identity,
                    bias=bias_sb["bqk"][:, co:co + 1], scale=1.0 / WQK_SCALE)

            def qt_ap(g, qb):
                return dr3(qt[g][:, :], HW)[:, :, qb * 128:(qb + 1) * 128]

            # per-(query,image) true max (DVE path), qb < NDVE
            mp = redpool.tile([128, NDVE * NIMG], f32, tag="mp", name="mp")
            # per-(query-block,image) shifted exp sums (LSE path)
            sacc = redpool.tile([128, (QB - NDVE) * NIMG], f32, tag="sacc",
                                name="sacc")
            # qbk partials accumulate here across the image loop
            ps_qbk = psp.tile([128, 1024], f32, tag="ps", name="ps_qbk")
            qbk_sb = redpool.tile([128, QB], f32, tag="qbk", name="qbk")

            # per-qb softmax state (filled incrementally during image 7)
            X8 = redpool.tile([128, QB], f32, tag="X8", name="X8")
            X8b = redpool.tile([128, QB], f32, tag="X8b", name="X8b")
            EX = redpool.tile([128, QB], f32, tag="EX", name="EX")
            S1t = redpool.tile([128, QB], f32, tag="S1t", name="S1t")
            lns = redpool.tile([128, (QB - NDVE) * NIMG], f32, tag="lns",
                               name="lns")

            def ex_col(qb):
                c = slice(qb, qb + 1)
                nc.vector.tensor_add(X8b[:, c], X8[:, c], qbk_sb[:, c])
                nc.scalar.activation(EX[:, c], X8b[:, c], AF.Exp, bias=0.0,
                                     scale=SCALE / NIMG,
                                     accum_out=S1t[:, c])

            def finalize_lse():
                """One batched ln over all LSE accumulators (a single Ln
                keeps the activation-table thrash to one round trip),
                emitted early in image 7 so the table loads overlap the
                remaining score matmuls."""
                nq = QB - NDVE
                nc.scalar.activation(lns[:], sacc[:], AF.Ln, bias=0.0,
                                     scale=1.0)
                xl = redpool.tile([128, nq], f32, tag="xl", name="xl")
                nc.vector.tensor_reduce(
                    xl[:],
                    lns[:, :].rearrange("p (q i) -> p q i", q=nq, i=NIMG),
                    axis=AX.X, op=ALU.add)
                nc.vector.tensor_scalar(
                    X8[:, NDVE:], xl[:], scalar1=NIMG * LSE_SHIFT,
                    scalar2=1.0 / LSE_T, op0=ALU.add, op1=ALU.mult)
                for qb in range(NDVE, QB):
                    ex_col(qb)

            def finalize_dve(qb):
                c = slice(qb, qb + 1)
                nc.vector.tensor_reduce(
                    X8[:, c], mp[:, qb * NIMG:(qb + 1) * NIMG],
                    axis=AX.X, op=ALU.add)
                ex_col(qb)

            # ---- image loop: 8 paired score tiles each ----
            # DVE- and scalar-consumed query blocks alternate 1:1 so
            # neither consumer builds a backlog; image 7 puts the LSE
            # blocks first so its single batched ln (and the two
            # activation-table round trips it costs) overlaps the last
            # DVE blocks' scores.
            base_order = [x for pair in zip(range(NDVE), range(NDVE, QB))
                          for x in pair]
            last_order = list(range(NDVE, QB)) + list(range(NDVE))
            for img in range(NIMG):
                qb_order = last_order if img == NIMG - 1 else base_order
                # qbk partials: 1 query block per image (tiny, hides in
                # the loop); after image 7's, evacuate the full row.
                for gg in range(G):
                    nc.tensor.matmul(
                        ps_qbk[:, img:img + 1],
                        dr3(x8_sb[gg][:, :],
                            NCORES * HW)[:, :, img * 128:(img + 1) * 128],
                        dr3(hbk8_sb[gg][:, :], 1),
                        start=(gg == 0), stop=(gg == G - 1), perf_mode=DR)
                if img == NIMG - 1:
                    nc.scalar.activation(qbk_sb[:], ps_qbk[:, :QB],
                                         AF.Identity, bias=0.0,
                                         scale=float(NIMG) / HBK_SCALE)
                for pos, qb in enumerate(qb_order):
                    ps = psp.tile([128, 1024], f32, tag="ps", name="ps_sc")
                    for h in range(KH):
                        col0 = img * HW + h * 512
                        for gg in range(G):
                            nc.tensor.matmul(
                                ps[:, h * 512:(h + 1) * 512], qt_ap(gg, qb),
                                dr3(x8_sb[gg][:, :],
                                    NCORES * HW)[:, :, col0:col0 + 512],
                                start=(gg == 0), stop=(gg == G - 1),
                                perf_mode=DR)
                    if qb < NDVE:
                        col = qb * NIMG + img
                        nc.vector.tensor_reduce(
                            mp[:, col:col + 1], ps[:], axis=AX.X, op=ALU.max)
                    else:
                        scr = scrpool.tile([128, 1024], bf16, tag="scr",
                                           name="scr")
                        col = (qb - NDVE) * NIMG + img
                        nc.scalar.activation(
                            scr[:], ps[:], AF.Exp, bias=shift_col[:],
                            scale=LSE_T, accum_out=sacc[:, col:col + 1])
                    if img == NIMG - 1:
                        if pos == QB - NDVE - 1:
                            finalize_lse()
                        elif qb < NDVE:
                            finalize_dve(qb)

            # ---- final conv on UNGATED x (bf16) during the img7 drain ----
            y_sb = [qpool.tile([128, HW], bf16, tag=f"y{co}", name=f"y{co}")
                    for co in range(CB)]
            for co in range(CB):
                ps = psp.tile([128, 1024], f32, tag="ps", name="ps_y")
                for h in range(KH):
                    sl = slice(h * 512, (h + 1) * 512)
                    for ci in range(CB):
                        nc.tensor.matmul(
                            ps[:, sl], w6_sb[ci][:, co * 128:(co + 1) * 128],
                            x_sb[ci][:, sl],
                            start=(ci == 0), stop=(ci == CB - 1))
                nc.scalar.activation(y_sb[co][:], ps[:], AF.Identity,
                                     bias=0.0, scale=1.0)

            # ---- total + reciprocal ----
            ps_tot = psp.tile([128, 1024], f32, tag="ps", name="ps_tot")
            nc.tensor.matmul(ps_tot[:1, :QB], ones_col[:], S1t[:],
                             start=True, stop=True)
            tot = redpool.tile([1, 1], f32, tag="tot", name="tot")
            nc.vector.tensor_reduce(tot[:], ps_tot[:1, :QB],
                                    axis=AX.X, op=ALU.add)
            rcp = redpool.tile([1, 1], f32, tag="rcp", name="rcp")
            nc.vector.reciprocal(rcp[:], tot[:])

            # ---- broadcast EX[p, qb] -> B[:, qb*128+p] on the PE ----
            # transpose EX via an identity matmul, then 8 one-hot selector
            # matmuls replicate row qb across all partitions of block qb.
            # No DRAM bounce, no per-column DMAs.
            ps_t = psp.tile([128, 1024], f32, tag="ps", name="ps_t")
            nc.tensor.matmul(ps_t[:QB, :128], EX[:, :], idm_sb[:],
                             start=True, stop=True)
            ext_bf = redpool.tile([QB, 128], bf16, tag="ext_bf",
                                  name="ext_bf")
            nc.scalar.activation(ext_bf[:], ps_t[:QB, :128], AF.Identity,
                                 bias=0.0, scale=1.0)
            ps_b = psp.tile([128, 1024], f32, tag="ps", name="ps_b")
            for qb in range(QB):
                nc.tensor.matmul(ps_b[:, qb * 128:(qb + 1) * 128],
                                 selm_sb[:, qb * 128:(qb + 1) * 128],
                                 ext_bf[:], start=True, stop=True)
            ps_rb = psp.tile([128, 1024], f32, tag="ps", name="ps_rb")
            nc.tensor.matmul(ps_rb[:, :1], ones_row[:], rcp[:],
                             start=True, stop=True)
            rb = redpool.tile([128, 1], f32, tag="rb", name="rb")
            nc.vector.tensor_copy(out=rb[:], in_=ps_rb[:, :1])
            B_bf = redpool.tile([128, HW], bf16, tag="B_bf", name="B_bf")
            nc.scalar.activation(B_bf[:, :], ps_b[:], AF.Identity, bias=0.0,
                                 scale=rb[:])

            # ---- gate y, add b6, write bf16 out (all on the DVE; the
            # tensor_scalar add runs in 4x mode) ----
            oengs = (nc.sync, nc.gpsimd)
            for co in range(CB):
                og = qpool.tile([128, HW], bf16, tag=f"og{co}",
                                name=f"og{co}")
                nc.vector.tensor_mul(og[:], y_sb[co][:], B_bf[:])
                o = outpool.tile([128, HW], bf16, tag=f"o{co}", name=f"o{co}")
                nc.vector.tensor_scalar(
                    o[:], og[:], scalar1=bias_sb["b6"][:, co:co + 1],
                    scalar2=None, op0=ALU.add)
                oengs[co % 2].dma_start(
                    out=out_ext[co * 128:(co + 1) * 128, :], in_=o[:])

    nc.compile()
    return nc


_BUILT = {}


def _get_nc():
    if "nc" not in _BUILT:
        _BUILT["nc"] = build_kernel()
    return _BUILT["nc"]


def make_in_maps(x, Wq, bq, Wk, bk, W6, b6):
    import ml_dtypes
    e4 = ml_dtypes.float8_e4m3
    bfl = ml_dtypes.bfloat16
    x = np.asarray(x, dtype=np.float32).reshape(B, C, HW)
    Wq = np.asarray(Wq, np.float32)
    Wk = np.asarray(Wk, np.float32)
    bq = np.asarray(bq, np.float32)
    bk = np.asarray(bk, np.float32)
    w6t = np.ascontiguousarray(np.asarray(W6, np.float32).T).astype(bfl)
    b6c = np.ascontiguousarray(np.asarray(b6, np.float32).reshape(C, 1))

    # host-folded score factorization
    Wqk = Wk.T @ Wq                       # [c_tilde, c_in]
    bqk = (Wk.T @ bq).reshape(C, 1)
    hbk = (Wq.T @ bk).reshape(C, 1)

    def dr_pack(ws):
        """[c(contraction), M] fp32 -> G x [128, 2*M] fp8 plane-paired."""
        M = ws.shape[1]
        w8 = ws.astype(e4).reshape(G, 2, 128, M)
        return [np.ascontiguousarray(
            np.transpose(w8[g], (1, 0, 2)).reshape(128, 2 * M))
            for g in range(G)]

    wqk8g = dr_pack(Wqk.T * WQK_SCALE)    # lhsT layout [c_in, c_tilde]
    hbk8g = dr_pack(hbk * HBK_SCALE)      # [c_in, 1]
    idm_h = np.eye(128, dtype=np.float32)
    selm_h = np.zeros((QB, QB * 128), dtype=bfl)
    for qb in range(QB):
        selm_h[qb, qb * 128:(qb + 1) * 128] = 1
    # fp8 DoubleRow layouts of x for every core, image-rolled so slot 0 is
    # the core's own image: c = g*256 + i*128 + p
    xc = np.transpose(x, (1, 0, 2))                      # [c, img, hw]
    x8_f = xc.astype(e4)
    maps = []
    for b in range(B):
        order = [(b + i) % B for i in range(B)]
        xr = x8_f[:, order, :].reshape(G, 2, 128, B * HW)
        x8g = [np.ascontiguousarray(
            np.transpose(xr[g], (1, 0, 2)).reshape(128, 2 * B * HW))
            for g in range(G)]
        m = {"x": np.ascontiguousarray(x[b]).astype(bfl),
             "w6t": w6t, "bqk": bqk, "b6": b6c, "idm": idm_h,
             "selm": selm_h}
        for g in range(G):
            m[f"x8g{g}"] = x8g[g]
            m[f"wqk8g{g}"] = wqk8g[g]
            m[f"hbk8g{g}"] = hbk8g[g]
        maps.append(m)
    return maps


def kernel(x, Wq, bq, Wk, bk, W6, b6, _trace=False):
    from concourse import bass_utils
    nc = _get_nc()
    in_maps = make_in_maps(x, Wq, bq, Wk, bk, W6, b6)
    res = bass_utils.run_bass_kernel_spmd(
        nc, in_maps, core_ids=list(range(NCORES)), trace=_trace)
    out = np.stack([np.asarray(res.results[i]["out"]) for i in range(NCORES)])
    out = out.reshape(B, C, H, W).astype(np.float32)
    if _trace:
        return out, res
    return out


# revision 32
# speedup vs baseline: 1.2047x; 1.0062x over previous
"""Trainium2 Bass kernel for nn_AllAttLayer (cross-batch attention gating layer).

Reference computation (B=8, C=512, H=W=32, HW=1024):
    xf = x as [B, HW, C]
    q = xf @ Wq.T + bq ; k = xf @ Wk.T + bk
    scores = q.flat @ k.flat.T                  # [B*HW, B*HW]
    xw = max over each image's keys, mean over images   # [B*HW]
    xw = softmax(xw * C**-0.5 per image)        # [B, HW]
    out = (x * xw) @ W6.T + b6  (1x1 conv)      # == W6 @ (x * xw)

Key algebraic restructure: scores = q^T k = q^T (Wk x + bk)
  = (Wk^T Wq x + Wk^T bq)^T x  +  (x^T Wq^T bk + bq^T bk).
The host folds Wqk := Wk^T Wq and bqk := Wk^T bq, so the kernel projects
each core's queries ONCE (qt = Wqk x + bqk, fp8 DoubleRow from the
replicated fp8 x) and the score matmuls consume the replicated fp8 x
DIRECTLY as the moving operand -- no per-image key projection, no key
evacuations. The k-bias term x^T(Wq^T bk) =: qbk is a 1-column fp8
projection (host-folded hbk := Wq^T bk) added to the logits; its
constant part bq^T bk is uniform over all queries and cancels in
softmax.

Sharding: core b owns image b (its 1024 queries). No collectives: the
host replicates x in fp8 DoubleRow layout, ROLLED per core so the
core's own image is slot 0 (the kernel is SPMD -- same program, per-core
data). Everything is c-major ([C, HW]) so no transposes are needed.

Engine schedule: per (query-block, image) the two 512-key score halves
land in one paired [128,1024] PSUM tile (2 banks); query blocks 0..NDVE-1
are consumed by a single DVE max-reduce (~1.19us), the rest by a
ScalarE exp-accumulate (LSE max approximation with temperature 2 and a
-80 shift to keep exp sums in range; the ~ln(n_eff)/2 overestimate is
~0.5 on logits*SCALE/8 ~ 0.003, well under the tolerance). Per image:
PE 7.6us of score matmuls vs DVE ~6.0us + scalar ~3.5us -- PE-paced.
The final conv runs bf16 on ungated x DURING the last image's score
drain; the gate (and b6) applies at the tail. fp32 elsewhere.
"""

import sys
import numpy as np

for _p in ("/opt/trn_rl_repo",):
    if _p not in sys.path:
        sys.path.insert(0, _p)

B, C, H, W = 8, 512, 32, 32
HW = H * W              # 1024 pixels per image
NCORES = 8
CB = C // 128           # 4 channel blocks
G = 2                   # DoubleRow groups (K=256 each)
QB = HW // 128          # 8 query blocks per core
KH = 2                  # key halves (512 keys each)
NIMG = NCORES
SCALE = 1.0 / float(np.sqrt(C))

WQK_SCALE = 64.0        # host scales Wqk by this before fp8
HBK_SCALE = 16.0        # host scales hbk by this before fp8
NDVE = 4                # query blocks per image consumed by DVE (rest: LSE)
LSE_T = 2.0             # LSE temperature
LSE_SHIFT = 36.0        # exp(t*s-SHIFT): sums stay in the HW ln
                        # spline's valid range [1e-18, 1e19]


def build_kernel(with_qbk=False):
    from concourse import bacc, tile, mybir

    f32 = mybir.dt.float32
    bf16 = mybir.dt.bfloat16
    fp8 = mybir.dt.float8e4
    DR = mybir.MatmulPerfMode.DoubleRow

    nc = bacc.Bacc("TRN2", target_bir_lowering=False, debug=False,
                   num_devices=NCORES)

    x_in = nc.dram_tensor("x", [C, HW], bf16, kind="ExternalInput").ap()
    w6t_in = nc.dram_tensor("w6t", [C, C], bf16, kind="ExternalInput").ap()
    x8_in = [nc.dram_tensor(f"x8g{g}", [128, 2 * NCORES * HW], fp8,
                            kind="ExternalInput").ap() for g in range(G)]
    wqk8_in = [nc.dram_tensor(f"wqk8g{g}", [128, 2 * C], fp8,
                              kind="ExternalInput").ap() for g in range(G)]
    if with_qbk:
        hbk8_in = [nc.dram_tensor(f"hbk8g{g}", [128, 2], fp8,
                                  kind="ExternalInput").ap()
                   for g in range(G)]
    bqk_in = nc.dram_tensor("bqk", [C, 1], f32, kind="ExternalInput").ap()
    idm_in = nc.dram_tensor("idm", [128, 128], f32, kind="ExternalInput").ap()
    selm_in = nc.dram_tensor("selm", [QB, QB * 128], bf16,
                             kind="ExternalInput").ap()
    b6_in = nc.dram_tensor("b6", [C, 1], f32, kind="ExternalInput").ap()
    out_ext = nc.dram_tensor("out", [C, HW], bf16,
                             kind="ExternalOutput").ap()

    AF = mybir.ActivationFunctionType
    ALU = mybir.AluOpType
    AX = mybir.AxisListType

    def dr3(ap, span):
        """[128, G*span] tile AP -> [128, 2, span] DoubleRow view."""
        return ap.rearrange("p (i n) -> p i n", i=2, n=span)

    with tile.TileContext(nc) as tc:
        with tc.tile_pool(name="consts", bufs=1) as consts, \
             tc.tile_pool(name="wpool", bufs=1) as wpool, \
             tc.tile_pool(name="xpool", bufs=1) as xpool, \
             tc.tile_pool(name="qpool", bufs=1) as qpool, \
             tc.tile_pool(name="redpool", bufs=1) as redpool, \
             tc.tile_pool(name="scrpool", bufs=3) as scrpool, \
             tc.tile_pool(name="outpool", bufs=1) as outpool, \
             tc.tile_pool(name="dram", bufs=1, space="DRAM") as dram, \
             tc.tile_pool(name="ps", bufs=4, space="PSUM") as psp:

            bias_sb = {}

            def load_bias(nm, src, eng):
                t = consts.tile([128, CB], f32, tag=f"{nm}_sb", name=f"{nm}_sb")
                for co in range(CB):
                    eng.dma_start(out=t[:, co:co + 1],
                                  in_=src[co * 128:(co + 1) * 128, :])
                bias_sb[nm] = t

            # ---- head loads ----
            # own-image x8 slices (slot 0) and Wqk first (they gate the
            # qt projection); the scalar queue gets no head DMAs.
            x8_sb = []
            for g in range(G):
                t = xpool.tile([128, 2 * NCORES * HW], fp8, tag=f"x8{g}",
                               name=f"x8{g}")
                x8_sb.append(t)
            wqk8_sb, hbk8_sb = [], []
            for g in range(G):
                t = wpool.tile([128, 2 * C], fp8, tag=f"wqk8{g}",
                               name=f"wqk8{g}")
                nc.sync.dma_start(out=t[:], in_=wqk8_in[g][:])
                wqk8_sb.append(t)
            for g in range(G):
                for i in range(2):
                    c0 = i * NCORES * HW
                    eng = nc.sync if (g + i) % 2 == 0 else nc.gpsimd
                    eng.dma_start(out=x8_sb[g][:, c0:c0 + HW],
                                  in_=x8_in[g][:, c0:c0 + HW])
            load_bias("bqk", bqk_in, nc.sync)
            if with_qbk:
                for g in range(G):
                    t = wpool.tile([128, 2], fp8, tag=f"hbk8{g}",
                                   name=f"hbk8{g}")
                    nc.gpsimd.dma_start(out=t[:], in_=hbk8_in[g][:])
                    hbk8_sb.append(t)
            for pair in range(4):
                for g in range(G):
                    for i in range(2):
                        c0 = i * NCORES * HW + max(pair * 2 * HW, HW)
                        c1 = i * NCORES * HW + (pair + 1) * 2 * HW
                        if c1 <= c0:
                            continue
                        eng = nc.sync if (g + i + pair) % 2 == 0 else nc.gpsimd
                        eng.dma_start(out=x8_sb[g][:, c0:c1],
                                      in_=x8_in[g][:, c0:c1])
            # tail-path inputs
            x_sb = []
            for ci in range(CB):
                t = xpool.tile([128, HW], bf16, tag=f"x{ci}", name=f"x{ci}")
                nc.sync.dma_start(out=t[:],
                                  in_=x_in[ci * 128:(ci + 1) * 128, :])
                x_sb.append(t)
            w6_sb = []
            for ci in range(CB):
                t = wpool.tile([128, C], bf16, tag=f"w6{ci}", name=f"w6{ci}")
                nc.gpsimd.dma_start(out=t[:],
                                    in_=w6t_in[ci * 128:(ci + 1) * 128, :])
                w6_sb.append(t)
            load_bias("b6", b6_in, nc.gpsimd)
            idm_sb = wpool.tile([128, 128], f32, tag="idm", name="idm")
            nc.sync.dma_start(out=idm_sb[:], in_=idm_in[:])
            selm_sb = wpool.tile([QB, QB * 128], bf16, tag="selm",
                                 name="selm")
            nc.gpsimd.dma_start(out=selm_sb[:], in_=selm_in[:])

            ones_col = consts.tile([128, 1], f32, tag="ones_col")
            nc.vector.memset(ones_col[:], 1.0)
            ones_row = consts.tile([1, 128], f32, tag="ones_row")
            nc.vector.memset(ones_row[:], 1.0)
            # warm the ln+exp+identity table set so no ACT_TABLE_LOAD lands
            # mid-pipeline (containment check keeps it resident)
            warm = consts.tile([1, 1], f32, tag="warm")
            nc.scalar.activation(warm[:], ones_col[:1, :1], AF.Ln,
                                 bias=0.0, scale=1.0)
            nc.scalar.activation(warm[:], ones_col[:1, :1], AF.Exp,
                                 bias=0.0, scale=1.0)
            shift_col = consts.tile([128, 1], f32, tag="shift_col")
            nc.vector.memset(shift_col[:], -LSE_SHIFT)

            # ---- qt = Wqk @ x_own + bqk, fp8 DR, plane-paired over c ----
            qt = [qpool.tile([128, G * HW], fp8, tag=f"qt{g}", name=f"qt{g}")
                  for g in range(G)]
            for co in range(CB):
                g, i = co // 2, co % 2
                ps = psp.tile([128, 1024], f32, tag="ps", name="ps_qt")
                for h in range(KH):
                    for gg in range(G):
                        nc.tensor.matmul(
                            ps[:, h * 512:(h + 1) * 512],
                            dr3(wqk8_sb[gg][:, :], C)[:, :,
                                                      co * 128:(co + 1) * 128],
                            dr3(x8_sb[gg][:, :],
                                NCORES * HW)[:, :, h * 512:(h + 1) * 512],
                            start=(gg == 0), stop=(gg == G - 1), perf_mode=DR)
                nc.scalar.activation(
                    qt[g][:, i * HW:(i + 1) * HW], ps[:], AF.Identity,
                    bias=bias_sb["bqk"][:, co:co + 1], scale=1.0 / WQK_SCALE)

            def qt_ap(g, qb):
                return dr3(qt[g][:, :], HW)[:, :, qb * 128:(qb + 1) * 128]

            # per-(query,image) true max (DVE path), qb < NDVE
            mp = redpool.tile([128, NDVE * NIMG], f32, tag="mp", name="mp")
            # per-(query-block,image) shifted exp sums (LSE path)
            sacc = redpool.tile([128, (QB - NDVE) * NIMG], f32, tag="sacc",
                                name="sacc")
            # qbk partials accumulate here across the image loop
            if with_qbk:
                ps_qbk = psp.tile([128, 1024], f32, tag="ps", name="ps_qbk")
                qbk_sb = redpool.tile([128, QB], f32, tag="qbk", name="qbk")

            # per-qb softmax state (filled incrementally during image 7)
            X8 = redpool.tile([128, QB], f32, tag="X8", name="X8")
            X8b = redpool.tile([128, QB], f32, tag="X8b", name="X8b")
            EX = redpool.tile([128, QB], f32, tag="EX", name="EX")
            S1t = redpool.tile([128, QB], f32, tag="S1t", name="S1t")
            lns = redpool.tile([128, (QB - NDVE) * NIMG], f32, tag="lns",
                               name="lns")

            def ex_col(qb):
                c = slice(qb, qb + 1)
                src = X8
                if with_qbk:
                    nc.vector.tensor_add(X8b[:, c], X8[:, c], qbk_sb[:, c])
                    src = X8b
                nc.scalar.activation(EX[:, c], src[:, c], AF.Exp, bias=0.0,
                                     scale=SCALE / NIMG,
                                     accum_out=S1t[:, c])

            def finalize_lse():
                """One batched ln over all LSE accumulators (a single Ln
                keeps the activation-table thrash to one round trip),
                emitted early in image 7 so the table loads overlap the
                remaining score matmuls."""
                nq = QB - NDVE
                nc.scalar.activation(lns[:], sacc[:], AF.Ln, bias=0.0,
                                     scale=1.0)
                xl = redpool.tile([128, nq], f32, tag="xl", name="xl")
                nc.vector.tensor_reduce(
                    xl[:],
                    lns[:, :].rearrange("p (q i) -> p q i", q=nq, i=NIMG),
                    axis=AX.X, op=ALU.add)
                nc.vector.tensor_scalar(
                    X8[:, NDVE:], xl[:], scalar1=NIMG * LSE_SHIFT,
                    scalar2=1.0 / LSE_T, op0=ALU.add, op1=ALU.mult)
                for qb in range(NDVE, QB):
                    ex_col(qb)

            def finalize_dve(qb):
                c = slice(qb, qb + 1)
                nc.vector.tensor_reduce(
                    X8[:, c], mp[:, qb * NIMG:(qb + 1) * NIMG],
                    axis=AX.X, op=ALU.add)
                ex_col(qb)

            # ---- image loop: 8 paired score tiles each ----
            # DVE- and scalar-consumed query blocks alternate 1:1 so
            # neither consumer builds a backlog; image 7 puts the LSE
            # blocks first so its single batched ln (and the two
            # activation-table round trips it costs) overlaps the last
            # DVE blocks' scores.
            base_order = [x for pair in zip(range(NDVE), range(NDVE, QB))
                          for x in pair]
            last_order = list(range(NDVE, QB)) + list(range(NDVE))
            for img in range(NIMG):
                qb_order = last_order if img == NIMG - 1 else base_order
                # qbk partials: 1 query block per image (tiny, hides in
                # the loop); after image 7's, evacuate the full row.
                if with_qbk:
                    for gg in range(G):
                        nc.tensor.matmul(
                            ps_qbk[:, img:img + 1],
                            dr3(x8_sb[gg][:, :],
                                NCORES * HW)[:, :, img * 128:(img + 1) * 128],
                            dr3(hbk8_sb[gg][:, :], 1),
                            start=(gg == 0), stop=(gg == G - 1),
                            perf_mode=DR)
                    if img == NIMG - 1:
                        nc.scalar.activation(qbk_sb[:], ps_qbk[:, :QB],
                                             AF.Identity, bias=0.0,
                                             scale=float(NIMG) / HBK_SCALE)
                for pos, qb in enumerate(qb_order):
                    ps = psp.tile([128, 1024], f32, tag="ps", name="ps_sc")
                    for h in range(KH):
                        col0 = img * HW + h * 512
                        for gg in range(G):
                            nc.tensor.matmul(
                                ps[:, h * 512:(h + 1) * 512], qt_ap(gg, qb),
                                dr3(x8_sb[gg][:, :],
                                    NCORES * HW)[:, :, col0:col0 + 512],
                                start=(gg == 0), stop=(gg == G - 1),
                                perf_mode=DR)
                    if qb < NDVE:
                        col = qb * NIMG + img
                        nc.vector.tensor_reduce(
                            mp[:, col:col + 1], ps[:], axis=AX.X, op=ALU.max)
                    else:
                        scr = scrpool.tile([128, 1024], bf16, tag="scr",
                                           name="scr")
                        col = (qb - NDVE) * NIMG + img
                        nc.scalar.activation(
                            scr[:], ps[:], AF.Exp, bias=shift_col[:],
                            scale=LSE_T, accum_out=sacc[:, col:col + 1])
                    if img == NIMG - 1:
                        if pos == QB - NDVE - 1:
                            finalize_lse()
                        elif qb < NDVE:
                            finalize_dve(qb)

            # ---- final conv on UNGATED x (bf16) during the img7 drain ----
            y_sb = [qpool.tile([128, HW], bf16, tag=f"y{co}", name=f"y{co}")
                    for co in range(CB)]
            for co in range(CB):
                ps = psp.tile([128, 1024], f32, tag="ps", name="ps_y")
                for h in range(KH):
                    sl = slice(h * 512, (h + 1) * 512)
                    for ci in range(CB):
                        nc.tensor.matmul(
                            ps[:, sl], w6_sb[ci][:, co * 128:(co + 1) * 128],
                            x_sb[ci][:, sl],
                            start=(ci == 0), stop=(ci == CB - 1))
                nc.scalar.activation(y_sb[co][:], ps[:], AF.Identity,
                                     bias=0.0, scale=1.0)

            # ---- total + reciprocal ----
            ps_tot = psp.tile([128, 1024], f32, tag="ps", name="ps_tot")
            nc.tensor.matmul(ps_tot[:1, :QB], ones_col[:], S1t[:],
                             start=True, stop=True)
            tot = redpool.tile([1, 1], f32, tag="tot", name="tot")
            nc.vector.tensor_reduce(tot[:], ps_tot[:1, :QB],
                                    axis=AX.X, op=ALU.add)
            rcp = redpool.tile([1, 1], f32, tag="rcp", name="rcp")
            nc.vector.reciprocal(rcp[:], tot[:])

            # ---- broadcast EX[p, qb] -> B[:, qb*128+p] on the PE ----
            # transpose EX via an identity matmul, then 8 one-hot selector
            # matmuls replicate row qb across all partitions of block qb.
            # No DRAM bounce, no per-column DMAs.
            ps_t = psp.tile([128, 1024], f32, tag="ps", name="ps_t")
            nc.tensor.matmul(ps_t[:QB, :128], EX[:, :], idm_sb[:],
                             start=True, stop=True)
            ext_bf = redpool.tile([QB, 128], bf16, tag="ext_bf",
                                  name="ext_bf")
            nc.scalar.activation(ext_bf[:], ps_t[:QB, :128], AF.Identity,
                                 bias=0.0, scale=1.0)
            ps_b = psp.tile([128, 1024], f32, tag="ps", name="ps_b")
            for qb in range(QB):
                nc.tensor.matmul(ps_b[:, qb * 128:(qb + 1) * 128],
                                 selm_sb[:, qb * 128:(qb + 1) * 128],
                                 ext_bf[:], start=True, stop=True)
            ps_rb = psp.tile([128, 1024], f32, tag="ps", name="ps_rb")
            nc.tensor.matmul(ps_rb[:, :1], ones_row[:], rcp[:],
                             start=True, stop=True)
            rb = redpool.tile([128, 1], f32, tag="rb", name="rb")
            nc.vector.tensor_copy(out=rb[:], in_=ps_rb[:, :1])
            B_bf = redpool.tile([128, HW], bf16, tag="B_bf", name="B_bf")
            nc.scalar.activation(B_bf[:, :], ps_b[:], AF.Identity, bias=0.0,
                                 scale=rb[:])

            # ---- gate y, add b6, write bf16 out (all on the DVE; the
            # tensor_scalar add runs in 4x mode) ----
            oengs = (nc.sync, nc.gpsimd)
            for co in range(CB):
                og = qpool.tile([128, HW], bf16, tag=f"og{co}",
                                name=f"og{co}")
                nc.vector.tensor_mul(og[:], y_sb[co][:], B_bf[:])
                o = outpool.tile([128, HW], bf16, tag=f"o{co}", name=f"o{co}")
                nc.vector.tensor_scalar(
                    o[:], og[:], scalar1=bias_sb["b6"][:, co:co + 1],
                    scalar2=None, op0=ALU.add)
                oengs[co % 2].dma_start(
                    out=out_ext[co * 128:(co + 1) * 128, :], in_=o[:])

    nc.compile()
    return nc


_BUILT = {}


def _get_nc(with_qbk=False):
    key = "qbk" if with_qbk else "plain"
    if key not in _BUILT:
        _BUILT[key] = build_kernel(with_qbk)
    return _BUILT[key]


def make_in_maps(x, Wq, bq, Wk, bk, W6, b6, with_qbk=False):
    import ml_dtypes
    e4 = ml_dtypes.float8_e4m3
    bfl = ml_dtypes.bfloat16
    x = np.asarray(x, dtype=np.float32).reshape(B, C, HW)
    Wq = np.asarray(Wq, np.float32)
    Wk = np.asarray(Wk, np.float32)
    bq = np.asarray(bq, np.float32)
    bk = np.asarray(bk, np.float32)
    w6t = np.ascontiguousarray(np.asarray(W6, np.float32).T).astype(bfl)
    b6c = np.ascontiguousarray(np.asarray(b6, np.float32).reshape(C, 1))

    # host-folded score factorization
    Wqk = Wk.T @ Wq                       # [c_tilde, c_in]
    bqk = (Wk.T @ bq).reshape(C, 1)
    hbk = (Wq.T @ bk).reshape(C, 1)

    def dr_pack(ws):
        """[c(contraction), M] fp32 -> G x [128, 2*M] fp8 plane-paired."""
        M = ws.shape[1]
        w8 = ws.astype(e4).reshape(G, 2, 128, M)
        return [np.ascontiguousarray(
            np.transpose(w8[g], (1, 0, 2)).reshape(128, 2 * M))
            for g in range(G)]

    wqk8g = dr_pack(Wqk.T * WQK_SCALE)    # lhsT layout [c_in, c_tilde]
    hbk8g = dr_pack(hbk * HBK_SCALE)      # [c_in, 1]
    idm_h = np.eye(128, dtype=np.float32)
    selm_h = np.zeros((QB, QB * 128), dtype=bfl)
    for qb in range(QB):
        selm_h[qb, qb * 128:(qb + 1) * 128] = 1
    # fp8 DoubleRow layouts of x for every core, image-rolled so slot 0 is
    # the core's own image: c = g*256 + i*128 + p
    xc = np.transpose(x, (1, 0, 2))                      # [c, img, hw]
    x8_f = xc.astype(e4)
    maps = []
    for b in range(B):
        order = [(b + i) % B for i in range(B)]
        xr = x8_f[:, order, :].reshape(G, 2, 128, B * HW)
        x8g = [np.ascontiguousarray(
            np.transpose(xr[g], (1, 0, 2)).reshape(128, 2 * B * HW))
            for g in range(G)]
        m = {"x": np.ascontiguousarray(x[b]).astype(bfl),
             "w6t": w6t, "bqk": bqk, "b6": b6c, "idm": idm_h,
             "selm": selm_h}
        for g in range(G):
            m[f"x8g{g}"] = x8g[g]
            m[f"wqk8g{g}"] = wqk8g[g]
            if with_qbk:
                m[f"hbk8g{g}"] = hbk8g[g]
        maps.append(m)
    return maps


def kernel(x, Wq, bq, Wk, bk, W6, b6, _trace=False):
    from concourse import bass_utils
    with_qbk = bool(np.any(np.asarray(bk, np.float32) != 0.0))
    nc = _get_nc(with_qbk)
    in_maps = make_in_maps(x, Wq, bq, Wk, bk, W6, b6, with_qbk)
    res = bass_utils.run_bass_kernel_spmd(
        nc, in_maps, core_ids=list(range(NCORES)), trace=_trace)
    out = np.stack([np.asarray(res.results[i]["out"]) for i in range(NCORES)])
    out = out.reshape(B, C, H, W).astype(np.float32)
    if _trace:
        return out, res
    return out
